# revision 30
# baseline (speedup 1.0000x reference)
"""Contrastive loss kernel for Trainium2, 8 NeuronCores (SPMD).

Math (matches the reference):
    z = concat(normalize(z_i), normalize(z_j))        # (2B, D) = (8192, 256)
    sim = (z @ z.T) / T
    positives[g] = sim[g, (g+B) mod 2B]               # (2B,)
    neg_max[g] = max_{j != g} sim[g, j]
    loss = mean(neg_max) - logsumexp(positives)       # scalar

v8 design (symmetric half-Gram + exp-space epilogue):
  Host pre-normalizes z (f64 norms), rolls per core, and ships the bf16
  TRANSPOSED operand zT[d, row] directly -- no on-device cast/transpose.

  Gram symmetry: core k computes only stationary j-chunks 0..39 (local bands
  0..4 = own band + next 4) against its own 1024 moving rows: 40 cells of
  [128 j, 1024 i] = 62.5% of the full Gram. Each computed cell credits its
  MOVING rows via an elementwise max-accumulate (i-side); cells whose j-rows
  are not credited elsewhere (local bands 1..3, cells 8..31) also need a
  row-collapse over the free axis (j-side). Band 0 (cells 0..7) contains
  both orderings internally; band 4 (cells 32..39) is computed by two cores
  (antipodal class), so its j-side is the partner's i-side.

  Epilogue engine split (measured: DVE f32-psum ops ~1.2us/cell, bf16
  elementwise 0.68us (2x mode), reduces always ~1.2us; ACT copy/exp 1.12us,
  +0.3us for its sum-accumulator read):
    cells 0..7  : DVE sub BIG on self-diag chunk, DVE max-acc psum->acc_raw.
    cells 8..31 : ACT activation Exp(80*sim) -> bf16 expcp + accumulator
                  sum_i exp = soft row-max (bias T2*ln(#near-max) ~ 0.008,
                  validated 1.9e-3 final rel err in fp-accurate emulation);
                  DVE bf16 max-acc expcp -> acc_exp (exact: max of exps).
    cells 32..39: two on the ACT-exp route (positives from the expcp
                  diagonal); the rest DVE max-acc psum->acc_raw, positives
                  extracted exactly from the f32 PSUM diagonal (ident mul +
                  reduce_sum). The split balances ACT vs DVE.
  Scheduling (the other half of the win): consts DMA'd first (diag-cell DVE
  ops gate on bigI), all input DMA interleaved on the sync queue (a single
  dma_start already fans out over all 16 DMA engines; the gpsimd SWDGE queue
  starts ~1us later), diag cells front-loaded to hide the ACT pipeline
  warmup, j-cells spread so ACT never backlogs PSUM recycling (psum pool
  4 x [128,1024] f32 = all 8 banks).
  Host (f64): 128-way partition maxes, T2*ln() back to sim units, cross-core
  row assembly, mean/LSE.
"""

import numpy as np

TEMPERATURE = 0.1
B, D = 4096, 256
R = 2 * B                # 8192 total rows
NCORES = 8
MROWS = R // NCORES      # 1024 rows per core
P = 128                  # SBUF partitions
NCELL = 40               # stationary j-chunks per core (5 bands)
NJROW = NCELL * P        # 5120 stationary rows shipped
KC = D // P              # 2 contraction chunks of 128
BIG = 30000.0            # diag mask subtrahend
T2 = 1.0 / 80.0          # exp-space temperature (scale=80)
# d4 cells routed via ACT-exp (rest take the direct-DVE raw route, with
# positives extracted exactly from PSUM); chosen to balance ACT vs DVE.
D4_EXP = frozenset((32, 33))

_CACHE = {}


def _host_constants():
    import ml_dtypes
    ident_bf = np.eye(P, dtype=np.float32).astype(ml_dtypes.bfloat16)
    bigI = (np.eye(P) * BIG).astype(np.float32)
    return {"ident_bf": ident_bf, "bigI": bigI}


def _build_nc():
    from contextlib import ExitStack

    import concourse.bass as bass
    import concourse.mybir as mybir
    import concourse.tile as tile
    from concourse import bacc

    f32 = mybir.dt.float32
    bf16 = mybir.dt.bfloat16
    X = mybir.AxisListType.X
    EXP = mybir.ActivationFunctionType.Exp

    nc = bacc.Bacc(
        "TRN2",
        target_bir_lowering=False,
        debug=False,
        enable_asserts=False,
        num_devices=NCORES,
    )

    zt0_dram = nc.dram_tensor("zt0", [P, NJROW], bf16, kind="ExternalInput")
    zt1_dram = nc.dram_tensor("zt1", [P, NJROW], bf16, kind="ExternalInput")
    ident_dram = nc.dram_tensor("ident_bf", [P, P], bf16, kind="ExternalInput")
    bigI_dram = nc.dram_tensor("bigI", [P, P], f32, kind="ExternalInput")
    accr_dram = nc.dram_tensor("acc_raw", [P, MROWS], bf16, kind="ExternalOutput")
    acce_dram = nc.dram_tensor("acc_exp", [P, MROWS], bf16, kind="ExternalOutput")
    esum_dram = nc.dram_tensor("expsum", [P, NCELL], f32, kind="ExternalOutput")
    pose_dram = nc.dram_tensor("pos_out", [P, MROWS // P], f32, kind="ExternalOutput")
    d39_dram = nc.dram_tensor("diag39", [P, P], bf16, kind="ExternalOutput")

    with tile.TileContext(nc) as tc, ExitStack() as ctx:
        singles = ctx.enter_context(tc.tile_pool(name="singles", bufs=1))
        exp_pool = ctx.enter_context(tc.tile_pool(name="exp_pool", bufs=8))
        scr_pool = ctx.enter_context(tc.tile_pool(name="scr_pool", bufs=2))
        psum = ctx.enter_context(
            tc.tile_pool(name="psum", bufs=4, space=bass.MemorySpace.PSUM)
        )

        # --- persistent buffers ---
        zt0 = singles.tile([P, NJROW], bf16)
        zt1 = singles.tile([P, NJROW], bf16)
        zT = [zt0, zt1]
        ident_bf = singles.tile([P, P], bf16)
        bigI = singles.tile([P, P], f32)
        acc_raw = singles.tile([P, MROWS], bf16)
        acc_exp = singles.tile([P, MROWS], bf16)
        expsum = singles.tile([P, NCELL], f32)
        pos_out = singles.tile([P, MROWS // P], f32)
        diag39 = singles.tile([P, P], bf16)

        # --- input DMA, ordered to feed the cell schedule: moving band
        # (cols 0..1023) first, then the first j-band (1024..2047), the d4
        # stationary (4096..5119), and the remaining j-bands; two queues ---
        B4 = 32 * P
        # early stream matches cell-0's exact operand order: zt0 halves,
        # then zt1 halves, then the small consts
        nc.sync.dma_start(out=zt0[:, :512], in_=zt0_dram.ap()[:, :512])
        nc.sync.dma_start(out=zt0[:, 512:MROWS], in_=zt0_dram.ap()[:, 512:MROWS])
        nc.sync.dma_start(out=zt1[:, :512], in_=zt1_dram.ap()[:, :512])
        nc.sync.dma_start(out=zt1[:, 512:MROWS], in_=zt1_dram.ap()[:, 512:MROWS])
        nc.sync.dma_start(out=ident_bf, in_=ident_dram.ap())
        nc.sync.dma_start(out=bigI, in_=bigI_dram.ap())
        for lo, hi in [(MROWS, 2 * MROWS), (B4, NJROW),
                       (2 * MROWS, 3 * MROWS), (3 * MROWS, B4)]:
            nc.sync.dma_start(out=zt0[:, lo:hi], in_=zt0_dram.ap()[:, lo:hi])
            nc.sync.dma_start(out=zt1[:, lo:hi], in_=zt1_dram.ap()[:, lo:hi])

        nc.vector.memset(acc_raw, -BIG)
        nc.vector.memset(acc_exp, 0.0)
        nc.vector.memset(expsum, 0.0)
        nc.vector.memset(pos_out, 0.0)

        def cell(s):
            o = s * P
            pp = psum.tile([P, MROWS], f32, name="pp")
            for c in range(KC):
                for u in range(MROWS // 512):
                    nc.tensor.matmul(
                        pp[:, u * 512:(u + 1) * 512],
                        zT[c][:, o:o + P],
                        zT[c][:, u * 512:(u + 1) * 512],
                        start=(c == 0),
                        stop=(c == KC - 1),
                    )
            if s < 8:
                # self-similarity diagonal at free offset 128*s
                nc.vector.tensor_sub(pp[:, o:o + P], pp[:, o:o + P], bigI)
                nc.vector.tensor_max(acc_raw, acc_raw, pp[:])
            elif s < 32:
                ec = exp_pool.tile([P, MROWS], bf16, name="ec")
                nc.scalar.activation(
                    ec, pp[:], EXP, scale=1.0 / T2,
                    accum_out=expsum[:, s:s + 1],
                )
                nc.vector.tensor_max(acc_exp, acc_exp, ec)
            elif s in D4_EXP:
                t = s - 32
                ec = exp_pool.tile([P, MROWS], bf16, name="ec")
                nc.scalar.activation(ec, pp[:], EXP, scale=1.0 / T2)
                nc.vector.tensor_max(acc_exp, acc_exp, ec)
                # positives: diagonal of chunk t (j = i + B in rolled coords)
                scr = scr_pool.tile([P, P], bf16, name="scr")
                nc.vector.tensor_mul(scr, ec[:, t * P:(t + 1) * P], ident_bf)
                nc.vector.reduce_sum(out=pos_out[:, t:t + 1], in_=scr, axis=X)
            else:
                t = s - 32
                nc.vector.tensor_max(acc_raw, acc_raw, pp[:])
                if s == 39:
                    # last cell: cheap chunk copy; host extracts the diagonal
                    nc.vector.tensor_copy(diag39, pp[:, t * P:(t + 1) * P])
                else:
                    # positives extracted exactly from the f32 PSUM diagonal
                    scr = scr_pool.tile([P, P], f32, name="scrf")
                    nc.vector.tensor_mul(scr, pp[:, t * P:(t + 1) * P], ident_bf)
                    nc.vector.reduce_sum(out=pos_out[:, t:t + 1], in_=scr, axis=X)

        # Schedule: front-load diag cells (first data to arrive, prompt DVE
        # consumers) to hide the ACT pipeline warmup, then spread j-cells so
        # ACT never backlogs; end on a DVE-raw d4 cell for a short tail.
        order = [0, 1, 2, 3, 4, 8, 9, 10, 5, 11, 12, 6, 13, 14, 7, 15]
        for t in range(7):
            order += [32 + t, 16 + 2 * t, 17 + 2 * t]
        order += [30, 31, 39]
        assert sorted(order) == list(range(NCELL))
        for s in order:
            cell(s)

        nc.sync.dma_start(out=pose_dram.ap(), in_=pos_out[:])
        nc.sync.dma_start(out=esum_dram.ap(), in_=expsum[:])
        nc.sync.dma_start(out=acce_dram.ap(), in_=acc_exp[:])
        nc.sync.dma_start(out=accr_dram.ap(), in_=acc_raw[:])
        nc.sync.dma_start(out=d39_dram.ap(), in_=diag39[:])

    nc.compile()
    return nc


def _get_nc():
    if "nc" not in _CACHE:
        _CACHE["nc"] = _build_nc()
    return _CACHE["nc"]


def _finish(results) -> np.ndarray:
    """Host epilogue in f64: partition maxes, ln back to sim units, assembly."""
    negmax = np.full(R, -np.inf)
    pos = np.empty(R)
    for k in range(NCORES):
        r = results[k]
        roll = (np.arange(R) + k * MROWS) % R  # local index -> global row
        acc_raw = np.asarray(r["acc_raw"], dtype=np.float64)
        acc_exp = np.asarray(r["acc_exp"], dtype=np.float64)
        expsum = np.asarray(r["expsum"], dtype=np.float64)
        pos_out = np.asarray(r["pos_out"], dtype=np.float64)
        d39 = np.asarray(r["diag39"], dtype=np.float64)
        pos_out[:, 7] = np.diagonal(d39)

        # own rows (local 0..1023): i-side credit
        own = np.maximum(
            acc_raw.max(axis=0),
            T2 * np.log(np.maximum(acc_exp.max(axis=0), 1e-300)),
        )
        g = roll[:MROWS]
        np.maximum.at(negmax, g, own)

        # j-side credit for cells 8..31 (local rows 1024..4095)
        jvals = T2 * np.log(np.maximum(expsum[:, 8:32], 1e-300))  # [128, 24]
        lrows = np.arange(8 * P, 32 * P)
        np.maximum.at(negmax, roll[lrows], jvals.T.reshape(-1))

        # positives for own rows: pos[g] = sim[g, g+B]; cols in D4_EXP hold
        # exp-space values, the rest raw sim
        pv = pos_out.copy()
        for s in D4_EXP:
            t = s - 32
            pv[:, t] = T2 * np.log(np.maximum(pv[:, t], 1e-300))
        pos[roll[:MROWS]] = pv.T.reshape(-1)

    negmax /= TEMPERATURE
    pos /= TEMPERATURE
    m = pos.max()
    lse = np.log(np.exp(pos - m).sum()) + m
    return np.array(negmax.mean() - lse, dtype=np.float32)


def kernel(z_i: np.ndarray, z_j: np.ndarray, _collect=None, _run_kwargs=None) -> np.ndarray:
    import ml_dtypes
    from concourse.bass_utils import run_bass_kernel_spmd

    z = np.concatenate(
        [np.asarray(z_i, np.float32), np.asarray(z_j, np.float32)], axis=0
    )
    inv = 1.0 / np.maximum(np.linalg.norm(z.astype(np.float64), axis=1), 1e-12)
    zhat = (z * inv[:, None].astype(np.float32)).astype(np.float32)
    zbf = zhat.astype(ml_dtypes.bfloat16)  # [R, D]
    consts = _host_constants()
    in_maps = []
    for k in range(NCORES):
        zk = np.roll(zbf, -k * MROWS, axis=0)[:NJROW]  # [5120, 256]
        zkT = np.ascontiguousarray(zk.T)               # [256, 5120]
        in_maps.append({
            "zt0": zkT[:P],
            "zt1": zkT[P:],
            **consts,
        })
    nc = _get_nc()
    res = run_bass_kernel_spmd(
        nc, in_maps, core_ids=list(range(NCORES)), **(_run_kwargs or {})
    )
    if _collect is not None:
        _collect.append(res)
    return _finish(res.results)


# revision 32
# speedup vs baseline: 1.1774x; 1.1774x over previous
"""Contrastive loss kernel for Trainium2, 8 NeuronCores (SPMD).

Math (matches the reference):
    z = concat(normalize(z_i), normalize(z_j))        # (2B, D) = (8192, 256)
    sim = (z @ z.T) / T
    positives[g] = sim[g, (g+B) mod 2B]               # (2B,)
    neg_max[g] = max_{j != g} sim[g, j]
    loss = mean(neg_max) - logsumexp(positives)       # scalar

v8 design (symmetric half-Gram + exp-space epilogue):
  Host pre-normalizes z (f64 norms), rolls per core, and ships the bf16
  TRANSPOSED operand zT[d, row] directly -- no on-device cast/transpose.

  Gram symmetry: core k computes only stationary j-chunks 0..39 (local bands
  0..4 = own band + next 4) against its own 1024 moving rows: 40 cells of
  [128 j, 1024 i] = 62.5% of the full Gram. Each computed cell credits its
  MOVING rows via an elementwise max-accumulate (i-side); cells whose j-rows
  are not credited elsewhere (local bands 1..3, cells 8..31) also need a
  row-collapse over the free axis (j-side). Band 0 (cells 0..7) contains
  both orderings internally; band 4 (cells 32..39) is computed by two cores
  (antipodal class), so its j-side is the partner's i-side.

  Epilogue engine split (measured: DVE f32-psum ops ~1.2us/cell, bf16
  elementwise 0.68us (2x mode), reduces always ~1.2us; ACT copy/exp 1.12us,
  +0.3us for its sum-accumulator read):
    cells 0..7  : DVE sub BIG on self-diag chunk, DVE max-acc psum->acc_raw.
    cells 8..31 : ACT activation Exp(80*sim) -> bf16 expcp + accumulator
                  sum_i exp = soft row-max (bias T2*ln(#near-max) ~ 0.008,
                  validated 1.9e-3 final rel err in fp-accurate emulation);
                  DVE bf16 max-acc expcp -> acc_exp (exact: max of exps).
    cells 32..39: two on the ACT-exp route (positives from the expcp
                  diagonal); the rest DVE max-acc psum->acc_raw, positives
                  extracted exactly from the f32 PSUM diagonal (ident mul +
                  reduce_sum). The split balances ACT vs DVE.
  Scheduling (the other half of the win): consts DMA'd first (diag-cell DVE
  ops gate on bigI), all input DMA interleaved on the sync queue (a single
  dma_start already fans out over all 16 DMA engines; the gpsimd SWDGE queue
  starts ~1us later), diag cells front-loaded to hide the ACT pipeline
  warmup, j-cells spread so ACT never backlogs PSUM recycling (psum pool
  4 x [128,1024] f32 = all 8 banks).
  Host (f64): 128-way partition maxes, T2*ln() back to sim units, cross-core
  row assembly, mean/LSE.
"""

import numpy as np

TEMPERATURE = 0.1
B, D = 4096, 256
R = 2 * B                # 8192 total rows
NCORES = 8
MROWS = R // NCORES      # 1024 rows per core
P = 128                  # SBUF partitions
NCELL = 40               # stationary j-chunks per core (5 bands)
NJROW = NCELL * P        # 5120 stationary rows shipped
KC = D // P              # 2 contraction chunks of 128
BIG = 30000.0            # diag mask subtrahend
T2 = 1.0 / 80.0          # exp-space temperature (scale=80)
# d4 cells routed via ACT-exp (rest take the direct-DVE raw route, with
# positives extracted exactly from PSUM); chosen to balance ACT vs DVE.
D4_EXP = frozenset((32, 33))

_CACHE = {}


def _host_constants():
    import ml_dtypes
    ident_bf = np.eye(P, dtype=np.float32).astype(ml_dtypes.bfloat16)
    bigI = (np.eye(P) * BIG).astype(np.float32)
    return {"ident_bf": ident_bf, "bigI": bigI}


def _build_nc():
    from contextlib import ExitStack

    import concourse.bass as bass
    import concourse.mybir as mybir
    import concourse.tile as tile
    from concourse import bacc

    f32 = mybir.dt.float32
    bf16 = mybir.dt.bfloat16
    X = mybir.AxisListType.X
    EXP = mybir.ActivationFunctionType.Exp

    nc = bacc.Bacc(
        "TRN2",
        target_bir_lowering=False,
        debug=False,
        enable_asserts=False,
        num_devices=NCORES,
    )

    zt0_dram = nc.dram_tensor("zt0", [P, NJROW], bf16, kind="ExternalInput")
    zt1_dram = nc.dram_tensor("zt1", [P, NJROW], bf16, kind="ExternalInput")
    ident_dram = nc.dram_tensor("ident_bf", [P, P], bf16, kind="ExternalInput")
    bigI_dram = nc.dram_tensor("bigI", [P, P], f32, kind="ExternalInput")
    accr_dram = nc.dram_tensor("acc_raw", [P, MROWS], bf16, kind="ExternalOutput")
    acce_dram = nc.dram_tensor("acc_exp", [P, MROWS], bf16, kind="ExternalOutput")
    esum_dram = nc.dram_tensor("expsum", [P, NCELL], f32, kind="ExternalOutput")
    pose_dram = nc.dram_tensor("pos_out", [P, MROWS // P], f32, kind="ExternalOutput")
    d39_dram = nc.dram_tensor("diag39", [P, P], bf16, kind="ExternalOutput")

    with tile.TileContext(nc) as tc, ExitStack() as ctx:
        singles = ctx.enter_context(tc.tile_pool(name="singles", bufs=1))
        exp_pool = ctx.enter_context(tc.tile_pool(name="exp_pool", bufs=8))
        scr_pool = ctx.enter_context(tc.tile_pool(name="scr_pool", bufs=2))
        psum = ctx.enter_context(
            tc.tile_pool(name="psum", bufs=4, space=bass.MemorySpace.PSUM)
        )

        # --- persistent buffers ---
        zt0 = singles.tile([P, NJROW], bf16)
        zt1 = singles.tile([P, NJROW], bf16)
        zT = [zt0, zt1]
        ident_bf = singles.tile([P, P], bf16)
        bigI = singles.tile([P, P], f32)
        acc_raw = singles.tile([P, MROWS], bf16)
        acc_exp = singles.tile([P, MROWS], bf16)
        expsum = singles.tile([P, NCELL], f32)
        pos_out = singles.tile([P, MROWS // P], f32)
        diag39 = singles.tile([P, P], bf16)

        # --- input DMA, ordered to feed the cell schedule: moving band
        # (cols 0..1023) first, then the first j-band (1024..2047), the d4
        # stationary (4096..5119), and the remaining j-bands; two queues ---
        B4 = 32 * P
        # early stream matches cell-0's exact operand order: zt0 halves,
        # then zt1 halves, then the small consts
        nc.sync.dma_start(out=zt0[:, :512], in_=zt0_dram.ap()[:, :512])
        nc.sync.dma_start(out=zt0[:, 512:MROWS], in_=zt0_dram.ap()[:, 512:MROWS])
        nc.sync.dma_start(out=zt1[:, :512], in_=zt1_dram.ap()[:, :512])
        nc.sync.dma_start(out=zt1[:, 512:MROWS], in_=zt1_dram.ap()[:, 512:MROWS])
        nc.sync.dma_start(out=ident_bf, in_=ident_dram.ap())
        nc.sync.dma_start(out=bigI, in_=bigI_dram.ap())
        for lo, hi in [(MROWS, 2 * MROWS), (B4, NJROW),
                       (2 * MROWS, 3 * MROWS), (3 * MROWS, B4)]:
            nc.sync.dma_start(out=zt0[:, lo:hi], in_=zt0_dram.ap()[:, lo:hi])
            nc.sync.dma_start(out=zt1[:, lo:hi], in_=zt1_dram.ap()[:, lo:hi])

        nc.vector.memset(acc_raw, -BIG)
        nc.vector.memset(acc_exp, 0.0)
        nc.vector.memset(expsum, 0.0)
        nc.vector.memset(pos_out, 0.0)

        def cell(s):
            o = s * P
            pp = psum.tile([P, MROWS], f32, name="pp")
            for c in range(KC):
                for u in range(MROWS // 512):
                    nc.tensor.matmul(
                        pp[:, u * 512:(u + 1) * 512],
                        zT[c][:, o:o + P],
                        zT[c][:, u * 512:(u + 1) * 512],
                        start=(c == 0),
                        stop=(c == KC - 1),
                    )
            if s < 8:
                # self-similarity diagonal at free offset 128*s
                nc.vector.tensor_sub(pp[:, o:o + P], pp[:, o:o + P], bigI)
                nc.vector.tensor_max(acc_raw, acc_raw, pp[:])
            elif s < 32:
                ec = exp_pool.tile([P, MROWS], bf16, name="ec")
                nc.scalar.activation(
                    ec, pp[:], EXP, scale=1.0 / T2,
                    accum_out=expsum[:, s:s + 1],
                )
                nc.vector.tensor_max(acc_exp, acc_exp, ec)
            elif s in D4_EXP:
                t = s - 32
                ec = exp_pool.tile([P, MROWS], bf16, name="ec")
                nc.scalar.activation(ec, pp[:], EXP, scale=1.0 / T2)
                nc.vector.tensor_max(acc_exp, acc_exp, ec)
                # positives: diagonal of chunk t (j = i + B in rolled coords)
                scr = scr_pool.tile([P, P], bf16, name="scr")
                nc.vector.tensor_mul(scr, ec[:, t * P:(t + 1) * P], ident_bf)
                nc.vector.reduce_sum(out=pos_out[:, t:t + 1], in_=scr, axis=X)
            else:
                t = s - 32
                nc.vector.tensor_max(acc_raw, acc_raw, pp[:])
                if s == 39:
                    # last cell: cheap chunk copy; host extracts the diagonal
                    nc.vector.tensor_copy(diag39, pp[:, t * P:(t + 1) * P])
                else:
                    # positives extracted exactly from the f32 PSUM diagonal
                    scr = scr_pool.tile([P, P], f32, name="scrf")
                    nc.vector.tensor_mul(scr, pp[:, t * P:(t + 1) * P], ident_bf)
                    nc.vector.reduce_sum(out=pos_out[:, t:t + 1], in_=scr, axis=X)

        # Schedule: front-load diag cells (first data to arrive, prompt DVE
        # consumers) to hide the ACT pipeline warmup, then spread j-cells so
        # ACT never backlogs; end on a DVE-raw d4 cell for a short tail.
        order = [0, 1, 2, 3, 4, 8, 9, 10, 5, 11, 12, 6, 13, 14, 7, 15]
        for t in range(7):
            order += [32 + t, 16 + 2 * t, 17 + 2 * t]
        order += [30, 31, 39]
        assert sorted(order) == list(range(NCELL))
        for s in order:
            cell(s)

        nc.sync.dma_start(out=pose_dram.ap(), in_=pos_out[:])
        nc.sync.dma_start(out=esum_dram.ap(), in_=expsum[:])
        nc.sync.dma_start(out=acce_dram.ap(), in_=acc_exp[:])
        nc.sync.dma_start(out=accr_dram.ap(), in_=acc_raw[:])
        nc.sync.dma_start(out=d39_dram.ap(), in_=diag39[:])

    nc.compile()
    return nc


def _get_nc():
    if "nc" not in _CACHE:
        _CACHE["nc"] = _build_nc()
    return _CACHE["nc"]


def _finish(results) -> np.ndarray:
    """Host epilogue in f64: partition maxes, ln back to sim units, assembly."""
    negmax = np.full(R, -np.inf)
    pos = np.empty(R)
    for k in range(NCORES):
        r = results[k]
        roll = (np.arange(R) + k * MROWS) % R  # local index -> global row
        acc_raw = np.asarray(r["acc_raw"], dtype=np.float64)
        acc_exp = np.asarray(r["acc_exp"], dtype=np.float64)
        expsum = np.asarray(r["expsum"], dtype=np.float64)
        pos_out = np.asarray(r["pos_out"], dtype=np.float64)
        d39 = np.asarray(r["diag39"], dtype=np.float64)
        pos_out[:, 7] = np.diagonal(d39)

        # own rows (local 0..1023): i-side credit
        own = np.maximum(
            acc_raw.max(axis=0),
            T2 * np.log(np.maximum(acc_exp.max(axis=0), 1e-300)),
        )
        g = roll[:MROWS]
        np.maximum.at(negmax, g, own)

        # j-side credit for cells 8..31 (local rows 1024..4095)
        jvals = T2 * np.log(np.maximum(expsum[:, 8:32], 1e-300))  # [128, 24]
        lrows = np.arange(8 * P, 32 * P)
        np.maximum.at(negmax, roll[lrows], jvals.T.reshape(-1))

        # positives for own rows: pos[g] = sim[g, g+B]; cols in D4_EXP hold
        # exp-space values, the rest raw sim
        pv = pos_out.copy()
        for s in D4_EXP:
            t = s - 32
            pv[:, t] = T2 * np.log(np.maximum(pv[:, t], 1e-300))
        pos[roll[:MROWS]] = pv.T.reshape(-1)

    negmax /= TEMPERATURE
    pos /= TEMPERATURE
    m = pos.max()
    lse = np.log(np.exp(pos - m).sum()) + m
    return np.array(negmax.mean() - lse, dtype=np.float32)


def kernel(z_i: np.ndarray, z_j: np.ndarray, _collect=None, _run_kwargs=None) -> np.ndarray:
    import ml_dtypes
    from concourse.bass_utils import run_bass_kernel_spmd

    z = np.concatenate(
        [np.asarray(z_i, np.float32), np.asarray(z_j, np.float32)], axis=0
    )
    inv = 1.0 / np.maximum(np.linalg.norm(z.astype(np.float64), axis=1), 1e-12)
    zhat = (z * inv[:, None].astype(np.float32)).astype(np.float32)
    zbf = zhat.astype(ml_dtypes.bfloat16)  # [R, D]
    consts = _host_constants()
    in_maps = []
    for k in range(NCORES):
        zk = np.roll(zbf, -k * MROWS, axis=0)[:NJROW]  # [5120, 256]
        zkT = np.ascontiguousarray(zk.T)               # [256, 5120]
        in_maps.append({
            "zt0": zkT[:P],
            "zt1": zkT[P:],
            **consts,
        })
    nc = _get_nc()
    res = run_bass_kernel_spmd(
        nc, in_maps, core_ids=list(range(NCORES)), **(_run_kwargs or {})
    )
    if _collect is not None:
        _collect.append(res)
    return _finish(res.results)
